# revision 8
# baseline (speedup 1.0000x reference)
"""Boolean REWA attention head kernel for Trainium2 (8 NeuronCores).

Reference computation (per batch element b):
    sq  = sign(Q @ Wq)            [N, MB]   (+-1)
    sk  = sign(K @ Wk)            [N, MB]
    dot = sq @ sk.T               [N, N]
    attn = softmax((dot + MB) / (2*sqrt(MB)))      (== softmax(dot/32))
    Vp  = V @ Wv                  [N, DH]
    out = attn @ Vp               [N, DH]
    returns (out, attn)

Sharding: data-parallel over batch B=8, one batch element per NeuronCore.
Weights (Wq, Wk, Wv) replicated on every core.

Per-core pipeline:
  Phase A: transpose Q/K/V into [d, n] layout with PE transpose-mode
           (exact), project with float32r matmuls (full PE rate at N=512),
           hard-sign (ACT) into bf16 sq^T/sk^T [MB, N]; Vp[k, h] in bf16.
  Phase B: dot tiles [q,k] via bf16 matmuls (exact: +-1 operands, fp32
           accumulate), exp on ACT with fused row-sum, normalize on DVE,
           stream attn rows to DRAM.
  Phase C: recompute dot transposed [k,q], exp to bf16, accumulate
           out^T = Vp^T @ exp(dot^T) in PSUM across k; then transpose
           out^T back, scale rows by 1/rowsum, store out.
"""

import math
import numpy as np

P = 128
N = 2048      # sequence length (queries == keys)
D = 1024      # model dim
MB = 256      # projection bits
DH = 128      # head dim
B = 8         # batch == number of cores

NT = N // P       # 16 row tiles
DT = D // P       # 8 d-blocks
MT = MB // P      # 2 m-blocks
SPAN = 512
NS = N // SPAN    # 4 spans
TPS = SPAN // P   # 4 row-tiles per span

_CACHED = {}


def _build_program():
    import concourse.bacc as bacc
    import concourse.mybir as mybir
    import concourse.tile as tile
    from concourse.masks import make_identity

    f32 = mybir.dt.float32
    f32r = mybir.dt.float32r
    bf16 = mybir.dt.bfloat16
    AF = mybir.ActivationFunctionType

    nc = bacc.Bacc(trn_type="TRN2")
    Q = nc.dram_tensor("Q", [N, D], f32, kind="ExternalInput")
    K = nc.dram_tensor("K", [N, D], f32, kind="ExternalInput")
    V = nc.dram_tensor("V", [N, D], f32, kind="ExternalInput")
    Wq = nc.dram_tensor("Wq", [D, MB], f32, kind="ExternalInput")
    Wk = nc.dram_tensor("Wk", [D, MB], f32, kind="ExternalInput")
    Wv = nc.dram_tensor("Wv", [D, DH], f32, kind="ExternalInput")
    attn_d = nc.dram_tensor("attn", [N, N], f32, kind="ExternalOutput")
    out_d = nc.dram_tensor("out", [N, DH], f32, kind="ExternalOutput")

    with tile.TileContext(nc) as tc:
        with (
            tc.tile_pool(name="consts", bufs=1) as consts,
            tc.tile_pool(name="persist", bufs=1) as persist,
            tc.tile_pool(name="work", bufs=2) as work,
            tc.tile_pool(name="attnbuf", bufs=3) as attnbuf,
        ):
            ident = consts.tile([P, P], f32)
            make_identity(nc, ident)
            ident_b = consts.tile([P, P], bf16)
            make_identity(nc, ident_b)

            # Weights: [d_inner=128, d_outer, m] layout (d on partitions).
            # f32r matmul operands must be produced as float32r (walrus
            # checkMatmultFP32r), so stage via f32 then round-copy on DVE.
            wq_st = work.tile([P, DT, MB], f32, tag="wst")
            nc.sync.dma_start(wq_st, Wq[:, :].rearrange("(o p) m -> p o m", p=P))
            wq_sb = consts.tile([P, DT, MB], f32r)
            nc.vector.tensor_copy(wq_sb, wq_st)
            wk_st = work.tile([P, DT, MB], f32, tag="wst")
            nc.sync.dma_start(wk_st, Wk[:, :].rearrange("(o p) m -> p o m", p=P))
            wk_sb = consts.tile([P, DT, MB], f32r)
            nc.vector.tensor_copy(wk_sb, wk_st)
            wv_sb = consts.tile([P, DT, DH], bf16)
            nc.gpsimd.dma_start(wv_sb, Wv[:, :].rearrange("(o p) m -> p o m", p=P))

            # Persistent tensors
            sqT = persist.tile([P, MT, N], bf16, tag="sqT")
            skT = persist.tile([P, MT, N], bf16, tag="skT")
            vp_sb = persist.tile([P, NT, DH], bf16, tag="vp")
            recs = persist.tile([P, NT], f32, tag="recs")

            # ---------------- Phase A ----------------
            with tc.tile_pool(name="psA", bufs=2, space="PSUM") as psA:

                def stage_a(x_dram, w_sb, sT):
                    """sign((X @ W))^T -> sT [P, MT, N] bf16, via X^T blocks."""
                    for s in range(NS):
                        nat = work.tile([P, TPS, D], f32, tag="nat")
                        nc.sync.dma_start(
                            nat,
                            x_dram[s * SPAN:(s + 1) * SPAN, :].rearrange(
                                "(t p) d -> p t d", p=P
                            ),
                        )
                        xt = work.tile([P, DT, SPAN], f32r, tag="xt")
                        for db in range(DT):
                            tp = psA.tile([P, SPAN], f32, tag="tp")
                            for t in range(TPS):
                                nc.tensor.transpose(
                                    tp[:, t * P:(t + 1) * P],
                                    nat[:, t, db * P:(db + 1) * P],
                                    ident,
                                )
                            nc.vector.tensor_copy(xt[:, db, :], tp)
                        for m in range(MT):
                            pj = psA.tile([P, SPAN], f32, tag="pj")
                            for db in range(DT):
                                nc.tensor.matmul(
                                    pj,
                                    w_sb[:, db, m * P:(m + 1) * P],
                                    xt[:, db, :],
                                    start=(db == 0),
                                    stop=(db == DT - 1),
                                )
                            nc.scalar.sign(sT[:, m, s * SPAN:(s + 1) * SPAN], pj)

                stage_a(K, wk_sb, skT)
                stage_a(Q, wq_sb, sqT)

                # V: bf16 path -> Vp [k, h]
                for s in range(NS):
                    vnat = work.tile([P, TPS, D], bf16, tag="vnat")
                    nc.gpsimd.dma_start(
                        vnat,
                        V[s * SPAN:(s + 1) * SPAN, :].rearrange(
                            "(t p) d -> p t d", p=P
                        ),
                    )
                    vt = work.tile([P, DT, SPAN], bf16, tag="vt")
                    for db in range(DT):
                        tpb = psA.tile([P, SPAN], bf16, tag="tpb")
                        for t in range(TPS):
                            nc.tensor.transpose(
                                tpb[:, t * P:(t + 1) * P],
                                vnat[:, t, db * P:(db + 1) * P],
                                ident_b,
                            )
                        nc.vector.tensor_copy(vt[:, db, :], tpb)
                    for t in range(TPS):
                        vpp = psA.tile([P, DH], f32, tag="vpp")
                        for db in range(DT):
                            nc.tensor.matmul(
                                vpp,
                                vt[:, db, t * P:(t + 1) * P],
                                wv_sb[:, db, :],
                                start=(db == 0),
                                stop=(db == DT - 1),
                            )
                        nc.vector.tensor_copy(vp_sb[:, s * TPS + t, :], vpp)

            # ---------------- Phase B ----------------
            with tc.tile_pool(name="psB", bufs=2, space="PSUM") as psB:
                for qt in range(NT):
                    dps = psB.tile([P, N], f32, tag="dps")
                    for s in range(NS):
                        for m in range(MT):
                            nc.tensor.matmul(
                                dps[:, s * SPAN:(s + 1) * SPAN],
                                sqT[:, m, qt * P:(qt + 1) * P],
                                skT[:, m, s * SPAN:(s + 1) * SPAN],
                                start=(m == 0),
                                stop=(m == MT - 1),
                            )
                    rs = attnbuf.tile([P, 1], f32, tag="rs")
                    ea = attnbuf.tile([P, N], f32, tag="ea")
                    nc.scalar.activation(
                        ea, dps, AF.Exp, scale=1.0 / 32.0, accum_out=rs
                    )
                    nc.vector.reciprocal(recs[:, qt:qt + 1], rs)
                    nc.vector.tensor_scalar_mul(ea, ea, recs[:, qt:qt + 1])
                    nc.sync.dma_start(attn_d[qt * P:(qt + 1) * P, :], ea)

            # ---------------- Phase C ----------------
            with tc.tile_pool(name="psOT", bufs=1, space="PSUM") as psOT:
                outT = psOT.tile([P, N], f32, tag="outT")
                with tc.tile_pool(name="psC", bufs=2, space="PSUM") as psC:
                    for kt in range(NT):
                        for h in range(2):
                            dT = psC.tile([P, 2 * SPAN], f32, tag="dT")
                            for s2 in range(2):
                                s = h * 2 + s2
                                for m in range(MT):
                                    nc.tensor.matmul(
                                        dT[:, s2 * SPAN:(s2 + 1) * SPAN],
                                        skT[:, m, kt * P:(kt + 1) * P],
                                        sqT[:, m, s * SPAN:(s + 1) * SPAN],
                                        start=(m == 0),
                                        stop=(m == MT - 1),
                                    )
                            eT = work.tile([P, 2 * SPAN], bf16, tag="eT")
                            nc.scalar.activation(eT, dT, AF.Exp, scale=1.0 / 32.0)
                            for s2 in range(2):
                                s = h * 2 + s2
                                nc.tensor.matmul(
                                    outT[:, s * SPAN:(s + 1) * SPAN],
                                    vp_sb[:, kt, :],
                                    eT[:, s2 * SPAN:(s2 + 1) * SPAN],
                                    start=(kt == 0),
                                    stop=(kt == NT - 1),
                                )
                outT_sb = work.tile([P, N], f32, tag="outTsb")
                nc.vector.tensor_copy(outT_sb, outT)
                with tc.tile_pool(name="psC2", bufs=2, space="PSUM") as psC2:
                    for qt in range(NT):
                        tpq = psC2.tile([P, P], f32, tag="tpq")
                        nc.tensor.transpose(
                            tpq, outT_sb[:, qt * P:(qt + 1) * P], ident
                        )
                        ot = work.tile([P, DH], f32, tag="ot")
                        nc.vector.tensor_scalar_mul(
                            ot, tpq, recs[:, qt:qt + 1]
                        )
                        nc.sync.dma_start(out_d[qt * P:(qt + 1) * P, :], ot)

    nc.finalize()
    return nc


def _get_program():
    if "nc" not in _CACHED:
        _CACHED["nc"] = _build_program()
    return _CACHED["nc"]


def kernel(Q, K, V, Wq, Wk, Wv, _trace=False):
    from concourse.bass_utils import run_bass_kernel_spmd

    Q = np.ascontiguousarray(np.asarray(Q, dtype=np.float32))
    K = np.ascontiguousarray(np.asarray(K, dtype=np.float32))
    V = np.ascontiguousarray(np.asarray(V, dtype=np.float32))
    Wq = np.ascontiguousarray(np.asarray(Wq, dtype=np.float32))
    Wk = np.ascontiguousarray(np.asarray(Wk, dtype=np.float32))
    Wv = np.ascontiguousarray(np.asarray(Wv, dtype=np.float32))

    nc = _get_program()
    in_maps = [
        dict(Q=Q[b], K=K[b], V=V[b], Wq=Wq, Wk=Wk, Wv=Wv) for b in range(B)
    ]
    res = run_bass_kernel_spmd(nc, in_maps, list(range(B)), trace=_trace)
    out = np.stack([res.results[b]["out"] for b in range(B)])
    attn = np.stack([res.results[b]["attn"] for b in range(B)])
    if _trace:
        _CACHED["last_result"] = res
    return out, attn


if __name__ == "__main__":
    nc = _get_program()
    ninst = sum(len(b.instructions) for b in nc.m.functions[0].blocks)
    print(f"build ok, instructions: {ninst}")
